# revision 1
# baseline (speedup 1.0000x reference)
"""AttentionPooling (segment softmax-weighted pooling) on 8 TRN2 NeuronCores.

Math (reference):
    h = relu(x @ W1 + b1)            # [N, 128]
    s = h @ W2 + b2                  # [N, 1]
    w = softmax(s, axis=0)           # over ALL nodes
    out[b] = sum_{i: batch[i]==b} w[i] * x[i]     # [512, 256]

Distribution: batch is sorted, so nodes are sharded at segment boundaries
(64 contiguous segments per core).  Each core computes exp(s_i) for its
nodes, a weighted segment-sum of x into its 64 segments plus per-segment
exp-sums (one matmul with a one-hot*exp stationary operand), then one
scalar AllGather (+local sum) of the per-core exp-sums gives the global
softmax denominator used to normalize on-device.  Softmax is computed
unshifted (exp(s) without max subtraction): scores for this MLP are O(1),
far inside fp16/fp32 range.

Device layouts (per 128-node tile, everything in natural row-major):
    x_nat  [node_p, d_f]  fp16 -> segment matmul moving operand
    x_t    [d_p, node_f]  fp16 -> MLP layer-1 moving operand (host pre-transposed)
    L1: h^T[h_p, node_f] = W1[d,h].T-stationary @ x_t           (PSUM f32)
    L2: s[node_p, 1]     = hr^T[h, node128]-stationary @ W2[h,1]
    seg: pooled[seg_p, 257_f] += onehot_w[node, seg]-stationary @ [x_nat | 1]
         (col 256 accumulates per-segment exp-sums)
"""

import numpy as np

from concourse import bacc, mybir, tile
from concourse import bass_utils
from concourse.mybir import AluOpType, ActivationFunctionType as AFT

P = 128          # partitions / nodes per tile
D = 256          # feature dim
H = 128          # hidden dim
B = 512          # total segments
NCORES = 8
SEGS = B // NCORES   # segments per core
GROUP = 512      # nodes per L1 matmul group
SUPER = 4096     # nodes per DMA batch (2 MiB per stream)
STRIDE = D + 1   # x_nat SBUF row block: 256 features + ones column

F32 = mybir.dt.float32
F16 = mybir.dt.float16
I16 = mybir.dt.int16

_cache: dict[int, object] = {}


def _build(nshard: int, loop: int = 1, sim: bool = False, dma_only: bool = False, no_tail: bool = False, tune: tuple = ()):
    tn = {"hp": 3, "sp": 3, "xn": 4, "xt": 4, "hr": 3, "oh": 6, "ring": 0, "super": SUPER, "dve_relu": 0, "gms": 1, "ct": 0, "spin": 0}
    tn.update(dict(tune))
    key = (nshard, loop, sim, dma_only, no_tail, tuple(sorted(tn.items())))
    if key in _cache:
        return _cache[key]
    assert nshard % GROUP == 0
    ntiles = nshard // P

    nc = bacc.Bacc("TRN2", target_bir_lowering=False, debug=False,
                   num_devices=1 if sim else NCORES)

    x_nat = nc.dram_tensor("x_nat", [nshard, D], F16, kind="ExternalInput")
    x_t = nc.dram_tensor("x_t", [D, nshard], F16, kind="ExternalInput")
    bloc = nc.dram_tensor("bloc", [P, ntiles], F32, kind="ExternalInput")
    w1 = nc.dram_tensor("w1", [D, H], F16, kind="ExternalInput")
    w2 = nc.dram_tensor("w2", [H, 1], F16, kind="ExternalInput")
    b1c = nc.dram_tensor("b1c", [H, 1], F32, kind="ExternalInput")
    b2c = nc.dram_tensor("b2c", [P, 1], F32, kind="ExternalInput")
    sid = nc.dram_tensor("sid", [P, SEGS], F32, kind="ExternalInput")
    pooled = nc.dram_tensor("pooled", [SEGS, D], F32, kind="ExternalOutput")

    with tile.TileContext(nc) as tc:
        with (
            tc.tile_pool(name="const", bufs=1) as const,
            tc.tile_pool(name="xn_pool", bufs=tn["xn"]) as xn_pool,
            tc.tile_pool(name="xt_pool", bufs=tn["xt"]) as xt_pool,
            tc.tile_pool(name="hr_pool", bufs=tn["hr"]) as hr_pool,
            tc.tile_pool(name="e_pool", bufs=3) as e_pool,
            tc.tile_pool(name="oh_pool", bufs=tn["oh"]) as oh_pool,
            tc.tile_pool(name="tail", bufs=1) as tail,
            tc.tile_pool(name="hp_pool", bufs=6 if tn["spin"] else tn["hp"],
                         space="PSUM") as hp_pool,
            tc.tile_pool(name="sp_pool", bufs=tn["sp"], space="PSUM") as sp_pool,
            tc.tile_pool(name="acc_pool", bufs=1, space="PSUM") as acc_pool,
            tc.tile_pool(name="zp_pool", bufs=1, space="PSUM") as zp_pool,
            tc.tile_pool(name="dram", bufs=1, space="DRAM") as dram,
        ):
            # ---- constants ----
            iota_i = const.tile([P, SEGS], I16)
            nc.gpsimd.iota(iota_i[:], pattern=[[1, SEGS]], base=0,
                           channel_multiplier=0)
            seg_iota = const.tile([P, SEGS], F16)
            nc.vector.tensor_copy(seg_iota[:], iota_i[:])

            # W1 [256,128] stored as [128, 2*H]: two K-chunks side by side
            w1t = const.tile([P, 2 * H], F16, name="w1t2")
            nc.scalar.dma_start(
                w1t[:].rearrange("p (c h) -> p c h", c=2),
                w1.ap().rearrange("(c p) h -> p c h", c=2))
            w2t = const.tile([H, 1], F16)
            nc.scalar.dma_start(w2t[:], w2.ap())
            b1t = const.tile([H, 1], F32)
            nc.scalar.dma_start(b1t[:], b1c.ap())
            b2t = const.tile([P, 1], F32)
            nc.scalar.dma_start(b2t[:], b2c.ap())
            bloc_t = const.tile([P, ntiles], F32)
            nc.scalar.dma_start(bloc_t[:], bloc.ap())
            sidt = const.tile([P, SEGS], F32)
            nc.scalar.dma_start(sidt[:], sid.ap())
            ones64 = const.tile([SEGS, 1], F32)
            nc.vector.memset(ones64[:], 1.0)
            bc8 = const.tile([NCORES, SEGS], F32)
            nc.vector.memset(bc8[:], 1.0)

            # persistent accumulator: [seg, 256 features + expsum col];
            # col-tiled variant uses both PE array halves -> two partition
            # halves accumulate alternating tiles, merged in the tail
            acc = acc_pool.tile([2 * SEGS if tn["ct"] else SEGS, STRIDE], F32)

            sz = tn["super"]
            supers = []
            pos = 0
            taper_from = max(0, nshard - sz)
            while pos < nshard:
                limit = sz if pos < taper_from else GROUP
                sn = min(limit, nshard - pos)
                supers.append((pos, sn))
                pos += sn

            for rep in range(loop):
              for (n0, sn) in supers:
                  tps = sn // P   # tiles in this super
                  xn = xn_pool.tile([P, tps * STRIDE], F16, tag="xn")
                  xn_v = xn[:].rearrange("p (t c) -> p t c", c=STRIDE)
                  nc.sync.dma_start(
                      xn_v[:, :, 0:D],
                      x_nat.ap()[n0:n0 + sn, :].rearrange(
                          "(t p) d -> p t d", p=P))
                  if tn["gms"]:
                      nc.gpsimd.memset(xn_v[:, :, D:STRIDE], 1.0)
                  else:
                      nc.vector.memset(xn_v[:, :, D:STRIDE], 1.0)

                  xt = xt_pool.tile([P, 2 * sn], F16, tag="xt")
                  xt_eng = nc.scalar if tn["ring"] else nc.sync
                  xt_eng.dma_start(
                      xt[:].rearrange("p (h n) -> p h n", h=2),
                      x_t.ap()[:, n0:n0 + sn].rearrange(
                          "(h p) n -> p h n", h=2))

                  for g in range(0 if dma_only else sn // GROUP):
                      hp = hp_pool.tile([H, GROUP], F32)
                      nc.tensor.matmul(hp[:], w1t[:, 0:H],
                                       xt[:, g * GROUP:(g + 1) * GROUP],
                                       start=True, stop=False)
                      nc.tensor.matmul(hp[:], w1t[:, H:2 * H],
                                       xt[:, sn + g * GROUP:
                                          sn + (g + 1) * GROUP],
                                       start=False, stop=True)
                      hr = hr_pool.tile([H, GROUP], F16)
                      if tn["dve_relu"] == 2:
                          # split relu ACT/DVE: PE and ACT co-bind above the
                          # DMA floor, so shed half the relu onto DVE
                          half = GROUP // 2
                          nc.scalar.activation(hr[:, :half], hp[:, :half],
                                               AFT.Relu, bias=b1t[:])
                          nc.vector.tensor_scalar(
                              hr[:, half:], hp[:, half:], b1t[:], 0.0,
                              op0=AluOpType.add, op1=AluOpType.max)
                      elif tn["dve_relu"] == 1:
                          nc.vector.tensor_scalar(
                              hr[:], hp[:], b1t[:], 0.0,
                              op0=AluOpType.add, op1=AluOpType.max)
                      else:
                          nc.scalar.activation(hr[:], hp[:], AFT.Relu,
                                               bias=b1t[:])

                      if tn["spin"]:
                          # scores overwrite dead h-data in hp's tail columns:
                          # frees the sp pool's 3 PSUM banks for hp depth 6
                          sp = hp[:, GROUP - GROUP // P:GROUP]
                      else:
                          sp = sp_pool.tile([P, GROUP // P], F32, name="sp")[:]
                      for j in range(GROUP // P):
                          nc.tensor.matmul(sp[:, j:j + 1],
                                           hr[:, j * P:(j + 1) * P], w2t[:],
                                           start=True, stop=True)
                      et = e_pool.tile([P, GROUP // P], F32)
                      nc.scalar.activation(et[:], sp[:], AFT.Exp, bias=b2t[:])

                      for j in range(GROUP // P):
                          t_idx = n0 // P + g * (GROUP // P) + j
                          oh = oh_pool.tile([P, SEGS], F16)
                          nc.vector.tensor_scalar(
                              oh[:], seg_iota[:],
                              bloc_t[:, t_idx:t_idx + 1],
                              et[:, j:j + 1],
                              op0=AluOpType.is_equal, op1=AluOpType.mult)
                          if tn["ct"]:
                              pos = (t_idx % 2) * SEGS
                              nc.tensor.matmul(
                                  acc[pos:pos + SEGS, :], oh[:],
                                  xn_v[:, g * (GROUP // P) + j, :],
                                  start=(t_idx < 2), stop=(t_idx >= ntiles - 2),
                                  skip_group_check=True,
                                  tile_position=(0, pos))
                          else:
                              nc.tensor.matmul(
                                  acc[:], oh[:],
                                  xn_v[:, g * (GROUP // P) + j, :],
                                  start=(t_idx == 0), stop=(t_idx == ntiles - 1),
                                  skip_group_check=True)

              # ---- tail: global denominator + normalize ----
              if dma_only or no_tail:
                  osb0 = tail.tile([SEGS, D], F32, name="osb0")
                  nc.vector.memset(osb0[:], 0.0)
                  nc.sync.dma_start(pooled.ap(), osb0[:])
                  continue
              if tn["ct"]:
                  accs = tail.tile([2 * SEGS, STRIDE], F32, name="accs")
                  nc.vector.tensor_copy(accs[:], acc[:])
                  macc = zp_pool.tile([SEGS, STRIDE], F32, name="macc")
                  nc.tensor.matmul(macc[:], sidt[:], accs[:],
                                   start=True, stop=True)
                  racc = macc
              else:
                  racc = acc
              ecol = tail.tile([SEGS, 1], F32)
              nc.vector.tensor_copy(ecol[:], racc[:, D:STRIDE])
              zscr = zp_pool.tile([SEGS, 1], F32, name="zscr")
              nc.tensor.matmul(zscr[0:1, :], ones64[:], ecol[:], start=True,
                               stop=True)
              zsb = tail.tile([1, 8], F32)
              nc.vector.memset(zsb[:], 0.0)
              nc.vector.tensor_copy(zsb[:, 0:1], zscr[0:1, :])

              if sim:
                  zra = tail.tile([NCORES, 8], F32, name="zra")
                  nc.vector.memset(zra[:], 0.0)
                  nc.vector.tensor_copy(zra[0:1, 0:1], zsb[:, 0:1])
              else:
                  cin = dram.tile([1, 8], F32)
                  cout = dram.tile([NCORES, 8], F32)
                  nc.gpsimd.dma_start(cin[:], zsb[:])
                  nc.gpsimd.collective_compute(
                      "AllGather", AluOpType.bypass,
                      replica_groups=[list(range(NCORES))],
                      ins=[cin[:].opt()], outs=[cout[:].opt()])
                  zra = tail.tile([NCORES, 8], F32, name="zra")
                  nc.sync.dma_start(zra[:], cout[:])

              nc.tensor.matmul(zscr[:], bc8[:], zra[:, 0:1], start=True,
                               stop=True, skip_group_check=True)
              rz = tail.tile([SEGS, 1], F32)
              nc.vector.reciprocal(rz[:], zscr[:])

              osb = tail.tile([SEGS, D], F32)
              nc.vector.tensor_scalar(osb[:], racc[:, 0:D], rz[:], None,
                                      op0=AluOpType.mult)
              nc.sync.dma_start(pooled.ap(), osb[:])

    nc.compile()
    _cache[key] = nc
    return nc


def _prepare(x, batch, W1, b1, W2, b2):
    x = np.asarray(x, dtype=np.float32)
    batch = np.asarray(batch)
    if batch.ndim != 1:
        batch = batch.reshape(-1)
    if np.any(np.diff(batch) < 0):
        # reference semantics are permutation-invariant; our sharding
        # needs contiguous segment ranges
        order = np.argsort(batch, kind="stable")
        batch = batch[order]
        x = x[order]
    bounds = np.searchsorted(batch, np.arange(0, B + 1, SEGS))
    counts = np.diff(bounds)
    nshard = int(-(-max(int(counts.max()), 1) // GROUP) * GROUP)

    x16 = x.astype(np.float16)
    w1_16 = np.ascontiguousarray(np.asarray(W1, np.float32).astype(np.float16))
    w2_16 = np.ascontiguousarray(
        np.asarray(W2, np.float32).astype(np.float16).reshape(H, 1))
    b1_32 = np.ascontiguousarray(
        np.asarray(b1, np.float32).reshape(H, 1))
    b2_32 = np.full((P, 1), np.float32(np.asarray(b2).reshape(-1)[0]),
                    dtype=np.float32)

    in_maps = []
    for c in range(NCORES):
        r0, r1 = int(bounds[c]), int(bounds[c + 1])
        n = r1 - r0
        xs = np.zeros((nshard, D), np.float16)
        xs[:n] = x16[r0:r1]
        xt = np.ascontiguousarray(xs.T)
        bl = np.full((nshard,), -1.0, np.float32)
        bl[:n] = (np.asarray(batch[r0:r1], np.int64) - SEGS * c).astype(
            np.float32)
        blt = np.ascontiguousarray(bl.reshape(nshard // P, P).T)
        in_maps.append({
            "x_nat": xs, "x_t": xt, "bloc": blt,
            "w1": w1_16, "w2": w2_16, "b1c": b1_32, "b2c": b2_32,
            "sid": np.ascontiguousarray(
                np.vstack([np.eye(SEGS), np.eye(SEGS)]).astype(np.float32)),
        })
    return nshard, in_maps


def kernel(x, batch, num_segments, W1, b1, W2, b2):
    assert int(num_segments) == B
    nshard, in_maps = _prepare(x, batch, W1, b1, W2, b2)
    nc = _build(nshard)
    res = bass_utils.run_bass_kernel_spmd(
        nc, in_maps, core_ids=list(range(NCORES)))
    out = np.concatenate([r["pooled"] for r in res.results], axis=0)
    return np.ascontiguousarray(out.astype(np.float32))



# revision 21
# speedup vs baseline: 1.1611x; 1.1611x over previous
"""AttentionPooling (segment softmax-weighted pooling) on 8 TRN2 NeuronCores.

Math (reference):
    h = relu(x @ W1 + b1)            # [N, 128]
    s = h @ W2 + b2                  # [N, 1]
    w = softmax(s, axis=0)           # over ALL nodes
    out[b] = sum_{i: batch[i]==b} w[i] * x[i]     # [512, 256]

Distribution: batch is sorted, so nodes are sharded at segment boundaries
(64 contiguous segments per core).  Each core computes exp(s_i) for its
nodes, a weighted segment-sum of x into its 64 segments plus per-segment
exp-sums (one matmul with a one-hot*exp stationary operand), then one
scalar AllGather (+local sum) of the per-core exp-sums gives the global
softmax denominator used to normalize on-device.  Softmax is computed
unshifted (exp(s) without max subtraction): scores for this MLP are O(1),
far inside fp16/fp32 range.

Device layouts (per 128-node tile):
    x_nat  [node_p, d_f]  fp16 -> segment matmul moving operand.  Nodes
           map to partitions in BLOCKS (partition p owns rows
           [p*ntiles, (p+1)*ntiles)) so each DMA partition line is
           tps*512B contiguous (16 KiB) instead of 512 B.
    x_t    [d_p, node_f]  fp8-e3m4, host-scaled by 2 -> MLP layer-1
           moving operand (host pre-transposed + node-permuted to match
           the block map; b1 scaled, W2 descaled to compensate).
           e3m4's 4 mantissa bits keep the softmax-weight error ~8e-3
           (e4m3 gives 2.6e-2, over the 2e-2 gate); W1 stays fp16 as
           the stationary operand -- mixed-dtype matmul works on TRN2.
    L1: h^T[h_p, node_f] = W1[d,h]-stationary @ x_t        (PSUM f32)
    L2: s[node_p, 1]     = hr^T[h, node128]-stationary @ W2[h,1]
    seg: pooled[seg_p, 257_f] += onehot_w[node, seg]-stationary @ [x_nat | 1]
         (col 256 accumulates per-segment exp-sums)

Timing (hwloop proxy metric, 8-core SPMD): fp16 baseline 138.9 us ->
e3m4 x_t 124.5 -> +block DMA layout 116.4; fp8 DMA floor is 94.6.
Rejected by measurement: DoubleRow-e4m3 (precision), col-tiled seg
accumulation, double-bank seg accumulation, DMA queue splitting,
software-pipelined emission order (Tile scheduler already reorders).
"""

import numpy as np
import ml_dtypes

from concourse import bacc, mybir, tile
from concourse import bass_utils
from concourse.mybir import AluOpType, ActivationFunctionType as AFT

P = 128          # partitions / nodes per tile
D = 256          # feature dim
H = 128          # hidden dim
B = 512          # total segments
NCORES = 8
SEGS = B // NCORES   # segments per core
GROUP = 512      # nodes per L1 matmul group
SUPER = 4096     # nodes per DMA batch (2 MiB per stream)
STRIDE = D + 1   # x_nat SBUF row block: 256 features + ones column

F32 = mybir.dt.float32
F16 = mybir.dt.float16
F8 = mybir.dt.float8e4
I16 = mybir.dt.int16
NP_F8 = ml_dtypes.float8_e4m3

_cache: dict[tuple, object] = {}

# tuning knobs (shared default so _prepare/_build agree on dtypes)
TUNE_DEFAULT = {"hp": 3, "sp": 3, "xn": 4, "xt": 4, "hr": 3, "oh": 6,
                "ring": 0, "super": SUPER, "dve_relu": 0, "gms": 1,
                "ct": 0, "spin": 0, "f8": 3, "skew": 0, "blk": 1,
                "probe": 0, "acc2": 0}


def _build(nshard: int, loop: int = 1, sim: bool = False,
           dma_only: bool = False, no_tail: bool = False,
           hwloop: int = 0, tune: tuple = ()):
    tn = dict(TUNE_DEFAULT)
    tn.update(dict(tune))
    key = (nshard, loop, sim, dma_only, no_tail, hwloop,
           tuple(sorted(tn.items())))
    if key in _cache:
        return _cache[key]
    assert nshard % GROUP == 0
    ntiles = nshard // P
    f8 = tn["f8"]
    # f8: 0 = fp16 x_t/W1; 2 = e4m3 both + DoubleRow; 3 = e3m4 x_t (host
    # 2x-scaled) with fp16 W1 stationary (mixed-dtype plain matmuls)
    xt_dt = {0: F16, 1: F8, 2: F8, 3: mybir.dt.float8e3}[f8]
    w1_dt = F8 if f8 in (1, 2) else F16

    nc = bacc.Bacc("TRN2", target_bir_lowering=False, debug=False,
                   num_devices=1 if sim else NCORES)

    x_nat = nc.dram_tensor("x_nat", [nshard, D], F16, kind="ExternalInput")
    x_t = nc.dram_tensor("x_t", [D, nshard], xt_dt, kind="ExternalInput")
    bloc = nc.dram_tensor("bloc", [P, ntiles], F32, kind="ExternalInput")
    w1 = nc.dram_tensor("w1", [D, H], w1_dt, kind="ExternalInput")
    w2 = nc.dram_tensor("w2", [H, 1], F16, kind="ExternalInput")
    b1c = nc.dram_tensor("b1c", [H, 1], F32, kind="ExternalInput")
    b2c = nc.dram_tensor("b2c", [P, 1], F32, kind="ExternalInput")
    sid = nc.dram_tensor("sid", [P, SEGS], F32, kind="ExternalInput")
    pooled = nc.dram_tensor("pooled", [SEGS, D], F32, kind="ExternalOutput")

    with tile.TileContext(nc) as tc:
        with (
            tc.tile_pool(name="const", bufs=1) as const,
            tc.tile_pool(name="xn_pool", bufs=tn["xn"]) as xn_pool,
            tc.tile_pool(name="xt_pool", bufs=tn["xt"]) as xt_pool,
            tc.tile_pool(name="hr_pool", bufs=tn["hr"]) as hr_pool,
            tc.tile_pool(name="e_pool", bufs=3) as e_pool,
            tc.tile_pool(name="oh_pool", bufs=tn["oh"]) as oh_pool,
            tc.tile_pool(name="tail", bufs=1) as tail,
            tc.tile_pool(name="hp_pool", bufs=6 if tn["spin"] else tn["hp"],
                         space="PSUM") as hp_pool,
            tc.tile_pool(name="sp_pool", bufs=tn["sp"], space="PSUM") as sp_pool,
            tc.tile_pool(name="acc_pool", bufs=1, space="PSUM") as acc_pool,
            tc.tile_pool(name="zp_pool", bufs=1, space="PSUM") as zp_pool,
            tc.tile_pool(name="dram", bufs=1, space="DRAM") as dram,
        ):
            # ---- constants ----
            iota_i = const.tile([P, SEGS], I16)
            nc.gpsimd.iota(iota_i[:], pattern=[[1, SEGS]], base=0,
                           channel_multiplier=0)
            seg_iota = const.tile([P, SEGS], F16)
            nc.vector.tensor_copy(seg_iota[:], iota_i[:])

            # W1 [256,128] stored as [128, 2*H]: two K-chunks side by side
            w1t = const.tile([P, 2 * H], w1_dt, name="w1t2")
            nc.scalar.dma_start(
                w1t[:].rearrange("p (c h) -> p c h", c=2),
                w1.ap().rearrange("(c p) h -> p c h", c=2))
            w2t = const.tile([H, 1], F16)
            nc.scalar.dma_start(w2t[:], w2.ap())
            b1t = const.tile([H, 1], F32)
            nc.scalar.dma_start(b1t[:], b1c.ap())
            b2t = const.tile([P, 1], F32)
            nc.scalar.dma_start(b2t[:], b2c.ap())
            bloc_t = const.tile([P, ntiles], F32)
            nc.scalar.dma_start(bloc_t[:], bloc.ap())
            sidt = const.tile([P, SEGS], F32)
            nc.scalar.dma_start(sidt[:], sid.ap())
            ones64 = const.tile([SEGS, 1], F32)
            nc.vector.memset(ones64[:], 1.0)
            bc8 = const.tile([NCORES, SEGS], F32)
            nc.vector.memset(bc8[:], 1.0)

            # persistent accumulator: [seg, 256 features + expsum col];
            # col-tiled variant uses both PE array halves -> two partition
            # halves accumulate alternating tiles, merged in the tail
            acc = acc_pool.tile([2 * SEGS if tn["ct"] else SEGS, STRIDE], F32)
            # acc2: alternate even/odd tiles between two PSUM banks so
            # consecutive seg matmuls overlap fill/drain (same-bank
            # accumulation serializes the PE pipeline); merged in the tail
            accB = acc_pool.tile([SEGS, STRIDE], F32,
                                 name="accB") if tn["acc2"] else None

            sz = tn["super"]
            supers = []
            pos = 0
            taper_from = max(0, nshard - sz)
            while pos < nshard:
                limit = sz if pos < taper_from else GROUP
                sn = min(limit, nshard - pos)
                supers.append((pos, sn))
                pos += sn

            def emit_super(n0, sn):
                tps = sn // P   # tiles in this super
                t0 = n0 // P
                xn = xn_pool.tile([P, tps * STRIDE], F16, tag="xn")
                xn_v = xn[:].rearrange("p (t c) -> p t c", c=STRIDE)
                xn_eng = nc.gpsimd if (tn["ring"] == 3 and
                                       (t0 // (sn // P)) % 2) else nc.sync
                if tn["blk"]:
                    # block node->partition map: partition p owns rows
                    # [p*ntiles, (p+1)*ntiles); a super is a tile-range, so
                    # each partition line is tps*D contiguous bytes (16 KiB
                    # at tps=32) instead of 512 B -- much better DMA
                    # efficiency.  bloc/x_t are host-permuted to match.
                    xn_eng.dma_start(
                        xn_v[:, :, 0:D],
                        x_nat.ap().rearrange(
                            "(p t) d -> p t d", p=P)[:, t0:t0 + tps, :])
                else:
                    nc.sync.dma_start(
                        xn_v[:, :, 0:D],
                        x_nat.ap()[n0:n0 + sn, :].rearrange(
                            "(t p) d -> p t d", p=P))
                if tn["gms"]:
                    nc.gpsimd.memset(xn_v[:, :, D:STRIDE], 1.0)
                else:
                    nc.vector.memset(xn_v[:, :, D:STRIDE], 1.0)

                xt = xt_pool.tile([P, 2 * sn], xt_dt, tag="xt")
                xt_v = xt[:].rearrange("p (h n) -> p h n", h=2)
                xt_eng = {0: nc.sync, 1: nc.scalar,
                          2: nc.gpsimd, 3: nc.sync}[tn["ring"]]
                xt_eng.dma_start(
                    xt_v,
                    x_t.ap()[:, n0:n0 + sn].rearrange(
                        "(h p) n -> p h n", h=2))
                return xn_v, xt, xt_v

            def main_loop():
              # flat group list; each entry carries its super's tiles
              groups = []
              for (n0, sn) in supers:
                  tiles = [None]
                  for g in range(0 if dma_only else sn // GROUP):
                      groups.append((tiles, n0, sn, g))
              sup_iter = iter(supers)
              emitted = {}

              st = {}

              def A(i):
                  tiles, n0, sn, g = groups[i]
                  if tiles[0] is None:
                      tiles[0] = emit_super(*next(sup_iter))
                  xn_v, xt, xt_v = tiles[0]
                  hp = hp_pool.tile([H, GROUP], F32)
                  if f8 == 2:
                      nc.tensor.matmul(
                          hp[:],
                          w1t[:].rearrange("p (c h) -> p c h", c=2),
                          xt_v[:, :, g * GROUP:(g + 1) * GROUP],
                          start=True, stop=True,
                          perf_mode=mybir.MatmulPerfMode.DoubleRow)
                  else:
                      nc.tensor.matmul(hp[:], w1t[:, 0:H],
                                       xt[:, g * GROUP:(g + 1) * GROUP],
                                       start=True, stop=False)
                      nc.tensor.matmul(hp[:], w1t[:, H:2 * H],
                                       xt[:, sn + g * GROUP:
                                          sn + (g + 1) * GROUP],
                                       start=False, stop=True)
                  st[i] = {"hp": hp}

              def R(i):
                  hp = st[i]["hp"]
                  hr = hr_pool.tile([H, GROUP], F16)
                  if tn["dve_relu"] == 2:
                      # split relu ACT/DVE: PE and ACT co-bind above the
                      # DMA floor, so shed half the relu onto DVE
                      half = GROUP // 2
                      nc.scalar.activation(hr[:, :half], hp[:, :half],
                                           AFT.Relu, bias=b1t[:])
                      nc.vector.tensor_scalar(
                          hr[:, half:], hp[:, half:], b1t[:], 0.0,
                          op0=AluOpType.add, op1=AluOpType.max)
                  elif tn["dve_relu"] == 1:
                      nc.vector.tensor_scalar(
                          hr[:], hp[:], b1t[:], 0.0,
                          op0=AluOpType.add, op1=AluOpType.max)
                  else:
                      nc.scalar.activation(hr[:], hp[:], AFT.Relu,
                                           bias=b1t[:])
                  st[i]["hr"] = hr

              def Bq(i):
                  hr = st[i]["hr"]
                  if tn["spin"]:
                      # scores overwrite dead h-data in hp's tail columns:
                      # frees the sp pool's 3 PSUM banks for hp depth 6
                      sp = st[i]["hp"][:, GROUP - GROUP // P:GROUP]
                  else:
                      sp = sp_pool.tile([P, GROUP // P], F32, name="sp")[:]
                  for j in range(GROUP // P):
                      nc.tensor.matmul(sp[:, j:j + 1],
                                       hr[:, j * P:(j + 1) * P], w2t[:],
                                       start=True, stop=True)
                  st[i]["sp"] = sp

              def E(i):
                  et = e_pool.tile([P, GROUP // P], F32)
                  nc.scalar.activation(et[:], st[i]["sp"], AFT.Exp,
                                       bias=b2t[:])
                  st[i]["et"] = et

              def O(i):
                  _, n0, sn, g = groups[i]
                  et = st[i]["et"]
                  ohs = []
                  for j in range(GROUP // P):
                      t_idx = n0 // P + g * (GROUP // P) + j
                      oh = oh_pool.tile([P, SEGS], F16)
                      nc.vector.tensor_scalar(
                          oh[:], seg_iota[:],
                          bloc_t[:, t_idx:t_idx + 1],
                          et[:, j:j + 1],
                          op0=AluOpType.is_equal, op1=AluOpType.mult)
                      ohs.append(oh)
                  st[i]["ohs"] = ohs

              def S(i):
                  tiles, n0, sn, g = groups[i]
                  xn_v = tiles[0][0]
                  ohs = st[i].get("ohs")
                  for j in range(GROUP // P):
                      t_idx = n0 // P + g * (GROUP // P) + j
                      # probe mode: constant stationary (timing-only build)
                      oh = seg_iota if ohs is None else ohs[j]
                      if tn["ct"]:
                          pos = (t_idx % 2) * SEGS
                          nc.tensor.matmul(
                              acc[pos:pos + SEGS, :], oh[:],
                              xn_v[:, g * (GROUP // P) + j, :],
                              start=(t_idx < 2), stop=(t_idx >= ntiles - 2),
                              skip_group_check=True,
                              tile_position=(0, pos))
                      elif tn["acc2"]:
                          tgt = accB if (t_idx % 2) else acc
                          nc.tensor.matmul(
                              tgt[:], oh[:],
                              xn_v[:, g * (GROUP // P) + j, :],
                              start=(t_idx < 2), stop=(t_idx >= ntiles - 2),
                              skip_group_check=True)
                      else:
                          nc.tensor.matmul(
                              acc[:], oh[:],
                              xn_v[:, g * (GROUP // P) + j, :],
                              start=(t_idx == 0), stop=(t_idx == ntiles - 1),
                              skip_group_check=True)
                  del st[i]

              n = len(groups)
              if dma_only:
                  for _ in supers:
                      emit_super(*next(sup_iter))
                  return
              if tn["probe"] == 1:
                  # timing probe: L1 + seg matmuls only, no score chain
                  for i in range(n):
                      A(i); S(i)
                  return
              if tn["probe"] == 2:
                  # timing probe: skip only the L2 matmuls (exp reads hp)
                  for i in range(n):
                      A(i); R(i)
                      st[i]["sp"] = st[i]["hp"][:, 0:GROUP // P]
                      E(i); O(i); S(i)
                  return
              if not tn["skew"]:
                  for i in range(n):
                      A(i); R(i); Bq(i); E(i); O(i); S(i)
                  return
              # software-pipelined: PE order per group i becomes
              # [..., B(i), A(i+1), S(i), B(i+1), ...] so the ACT(exp) ->
              # DVE(onehot) chain of group i hides under A(i+1) instead of
              # blocking the strict-FIFO PE queue at S(i).
              A(0); R(0)
              for i in range(n):
                  Bq(i); E(i)
                  if i + 1 < n:
                      A(i + 1); R(i + 1)
                  O(i); S(i)

            def tail_part(fake_cc: bool):
              # ---- tail: global denominator + normalize ----
              if dma_only or no_tail:
                  osb0 = tail.tile([SEGS, D], F32, name="osb0")
                  nc.vector.memset(osb0[:], 0.0)
                  nc.sync.dma_start(pooled.ap(), osb0[:])
                  return
              if tn["ct"]:
                  accs = tail.tile([2 * SEGS, STRIDE], F32, name="accs")
                  nc.vector.tensor_copy(accs[:], acc[:])
                  macc = zp_pool.tile([SEGS, STRIDE], F32, name="macc")
                  nc.tensor.matmul(macc[:], sidt[:], accs[:],
                                   start=True, stop=True)
                  racc = macc
              elif tn["acc2"]:
                  mcp = tail.tile([SEGS, STRIDE], F32, name="mcp")
                  nc.vector.tensor_copy(mcp[:], acc[:])
                  msum = tail.tile([SEGS, STRIDE], F32, name="msum")
                  nc.vector.tensor_tensor(msum[:], mcp[:], accB[:],
                                          op=AluOpType.add)
                  racc = msum
              else:
                  racc = acc
              ecol = tail.tile([SEGS, 1], F32)
              nc.vector.tensor_copy(ecol[:], racc[:, D:STRIDE])
              zscr = zp_pool.tile([SEGS, 1], F32, name="zscr")
              nc.tensor.matmul(zscr[0:1, :], ones64[:], ecol[:], start=True,
                               stop=True)
              zsb = tail.tile([1, 8], F32)
              nc.vector.memset(zsb[:], 0.0)
              nc.vector.tensor_copy(zsb[:, 0:1], zscr[0:1, :])

              if sim or fake_cc:
                  zra = tail.tile([NCORES, 8], F32, name="zra")
                  nc.vector.memset(zra[:], 0.0)
                  nc.vector.tensor_copy(zra[0:1, 0:1], zsb[:, 0:1])
              else:
                  cin = dram.tile([1, 8], F32)
                  cout = dram.tile([NCORES, 8], F32)
                  nc.gpsimd.dma_start(cin[:], zsb[:])
                  nc.gpsimd.collective_compute(
                      "AllGather", AluOpType.bypass,
                      replica_groups=[list(range(NCORES))],
                      ins=[cin[:].opt()], outs=[cout[:].opt()])
                  zra = tail.tile([NCORES, 8], F32, name="zra")
                  nc.sync.dma_start(zra[:], cout[:])

              nc.tensor.matmul(zscr[:], bc8[:], zra[:, 0:1], start=True,
                               stop=True, skip_group_check=True)
              rz = tail.tile([SEGS, 1], F32)
              nc.vector.reciprocal(rz[:], zscr[:])

              osb = tail.tile([SEGS, D], F32)
              nc.vector.tensor_scalar(osb[:], racc[:, 0:D], rz[:], None,
                                      op0=AluOpType.mult)
              nc.sync.dma_start(pooled.ap(), osb[:])

            if hwloop:
                hints = (mybir.EngineType.PE, mybir.EngineType.Activation,
                         mybir.EngineType.DVE, mybir.EngineType.Pool,
                         mybir.EngineType.SP)
                with tc.For_i(0, hwloop, 1, hint_engines=hints):
                    main_loop()
                    tail_part(fake_cc=True)
            else:
                for rep in range(loop):
                    main_loop()
                    tail_part(fake_cc=False)

    nc.compile()
    _cache[key] = nc
    return nc


def _prepare(x, batch, W1, b1, W2, b2, f8: int = TUNE_DEFAULT["f8"],
             blk: int = TUNE_DEFAULT["blk"]):
    x = np.asarray(x, dtype=np.float32)
    batch = np.asarray(batch)
    if batch.ndim != 1:
        batch = batch.reshape(-1)
    if np.any(np.diff(batch) < 0):
        # reference semantics are permutation-invariant; our sharding
        # needs contiguous segment ranges
        order = np.argsort(batch, kind="stable")
        batch = batch[order]
        x = x[order]
    bounds = np.searchsorted(batch, np.arange(0, B + 1, SEGS))
    counts = np.diff(bounds)
    nshard = int(-(-max(int(counts.max()), 1) // GROUP) * GROUP)

    x16 = x.astype(np.float16)
    w1_f32 = np.asarray(W1, np.float32)
    w2_f32 = np.asarray(W2, np.float32).reshape(H, 1)
    if f8 == 3:
        # x_t host-scaled by 2 to keep N(0,1) data inside e3m4's narrow
        # range; relu is positively homogeneous, so b1 scales by 2 and
        # W2 descales to compensate.  W1 stays fp16 (stationary operand).
        w1_c = np.ascontiguousarray(w1_f32.astype(np.float16))
        w2_16 = np.ascontiguousarray((w2_f32 / 2.0).astype(np.float16))
        b1_32 = np.ascontiguousarray(
            (np.asarray(b1, np.float32) * 2.0).reshape(H, 1))
    elif f8:
        # scale W1 by 16 so fp8e4m3 is well-ranged for N(0,1/16) weights;
        # relu is positively homogeneous, so descale W2 to compensate
        w1_c = np.ascontiguousarray((w1_f32 * 16.0).astype(NP_F8))
        w2_16 = np.ascontiguousarray((w2_f32 / 16.0).astype(np.float16))
        b1_32 = np.ascontiguousarray(
            (np.asarray(b1, np.float32) * 16.0).reshape(H, 1))
    else:
        w1_c = np.ascontiguousarray(w1_f32.astype(np.float16))
        w2_16 = np.ascontiguousarray(w2_f32.astype(np.float16))
        b1_32 = np.ascontiguousarray(
            np.asarray(b1, np.float32).reshape(H, 1))
    b2_32 = np.full((P, 1), np.float32(np.asarray(b2).reshape(-1)[0]),
                    dtype=np.float32)

    in_maps = []
    for c in range(NCORES):
        r0, r1 = int(bounds[c]), int(bounds[c + 1])
        n = r1 - r0
        xs = np.zeros((nshard, D), np.float16)
        xs[:n] = x16[r0:r1]
        T = nshard // P
        if f8 == 3:
            xf = np.zeros((nshard, D), np.float32)
            xf[:n] = x[r0:r1] * 2.0
            if blk:
                xf = xf.reshape(P, T, D).transpose(1, 0, 2).reshape(nshard, D)
            xt = np.ascontiguousarray(xf.T.astype(ml_dtypes.float8_e3m4))
        else:
            xsp = xs
            if blk:
                xsp = xs.reshape(P, T, D).transpose(1, 0, 2).reshape(
                    nshard, D)
            if f8:
                xt = np.ascontiguousarray(xsp.T.astype(NP_F8))
            else:
                xt = np.ascontiguousarray(xsp.T)
        bl = np.full((nshard,), -1.0, np.float32)
        bl[:n] = (np.asarray(batch[r0:r1], np.int64) - SEGS * c).astype(
            np.float32)
        if blk:
            blt = np.ascontiguousarray(bl.reshape(P, T))
        else:
            blt = np.ascontiguousarray(bl.reshape(T, P).T)
        in_maps.append({
            "x_nat": xs, "x_t": xt, "bloc": blt,
            "w1": w1_c, "w2": w2_16, "b1c": b1_32, "b2c": b2_32,
            "sid": np.ascontiguousarray(
                np.vstack([np.eye(SEGS), np.eye(SEGS)]).astype(np.float32)),
        })
    return nshard, in_maps


def kernel(x, batch, num_segments, W1, b1, W2, b2):
    assert int(num_segments) == B
    nshard, in_maps = _prepare(x, batch, W1, b1, W2, b2)
    nc = _build(nshard)
    res = bass_utils.run_bass_kernel_spmd(
        nc, in_maps, core_ids=list(range(NCORES)))
    out = np.concatenate([r["pooled"] for r in res.results], axis=0)
    return np.ascontiguousarray(out.astype(np.float32))
